# revision 11
# baseline (speedup 1.0000x reference)
# Trainium2 Bass kernel for EnhancedGNNTransformerEncoder.
#
# Strategy (data-parallel over graphs, 4 graphs/core on 8 cores):
# The per-graph sparse TransformerConv attention (16 random in-edges per node,
# edges never cross graphs) is reformulated as dense per-graph 512x512
# attention with an edge-count mask A[src,dst] (host-precomputed from
# edge_index; exactly reproduces segment-softmax semantics incl. duplicate
# edges).  Everything on-device is dense linear algebra:
#   per layer:  Q/K/skip projections (feature-major), V projection
#   (node-major), per-(graph,head): Lt = Kt_h^T @ Qt_h on PE, exp on ScalarE,
#   count-mask multiply on VectorE, P @ [V|1] on PE (softmax denominators via
#   an appended ones column), reciprocal + PE-broadcast for the division,
#   skip-add on GpSimd, ReLU on ScalarE.  Then gnn_output Linear, per-graph
#   dense MHA (same pattern, no mask), out-proj fused with LayerNorm.
# Matmuls run as float32r (1 cycle/row, ~1e-4 rel err); accumulation fp32.
# Softmax max-subtraction is dropped: logits are O(10), exp cannot overflow,
# and normalization makes it mathematically identical.

import sys

for _p in ("/opt/trn_rl_repo", "/opt/trn_rl_repo/concourse"):
    if _p not in sys.path:
        sys.path.insert(0, _p)

import numpy as np

NG, NPG, DEG = 32, 512, 16
N, E = NG * NPG, NG * NPG * DEG
IN_CH, HID, HEADS, HDIM, OUT, NLAYERS = 128, 64, 8, 512, 256, 4
NCORES = 8
GPC = NG // NCORES          # graphs per core
NPC = GPC * NPG             # nodes per core
P = 128
NB = NPG // P               # 128-node blocks per graph
EPS = 1e-5

_cache = {}


def _build():
    import concourse.bass as bass  # noqa: F401
    import concourse.tile as tile
    from concourse import bacc, mybir
    from contextlib import ExitStack

    f32, f32r = mybir.dt.float32, mybir.dt.float32r
    AF = mybir.ActivationFunctionType
    OP = mybir.AluOpType

    nc = bacc.Bacc("TRN2", target_bir_lowering=False, debug=False,
                   num_devices=NCORES)

    def din(name, shape):
        return nc.dram_tensor(name, shape, f32, kind="ExternalInput").ap()

    xt_d = din("xt", [P, NPC])
    adj_d = din("adj", [GPC, 4, P, NPG])
    ones_d = din("ones", [P, P])
    wq_d, wk_d, wsk_d, wv_d, ball_d, bvrep_d = [], [], [], [], [], []
    for l in range(NLAYERS):
        C = IN_CH if l == 0 else HDIM
        wq_d.append(din(f"wq{l}", [C, HDIM]))
        wk_d.append(din(f"wk{l}", [C, HDIM]))
        wsk_d.append(din(f"wsk{l}", [C, HDIM]))
        wv_d.append(din(f"wv{l}", [C, HDIM]))
        ball_d.append(din(f"ball{l}", [P, 12]))
        bvrep_d.append(din(f"bvrep{l}", [P, HDIM]))
    wout_d = din("wout", [HDIM, OUT])
    bout_d = din("bout", [P, 2])
    wqkv_d = din("wqkv", [OUT, 3 * OUT])
    bqk_d = din("bqk", [P, 4])
    bvh_d = din("bvh", [P, OUT])
    wop_d = din("wop", [OUT, OUT])
    bop_d = din("bop", [P, OUT])
    lng_d = din("lng", [P, OUT])
    lnb_d = din("lnb", [P, OUT])
    out_d = nc.dram_tensor("out", [NPC, OUT], f32, kind="ExternalOutput").ap()

    with tile.TileContext(nc) as tc, ExitStack() as ctx:
        def pool(name, bufs, space="SBUF"):
            return ctx.enter_context(
                tc.tile_pool(name=name, bufs=bufs, space=space))

        hp = pool("hp", 1)       # H chunks, xt, GT
        wp = pool("wp", 1)       # layer weights
        cst = pool("cst", 1)     # constants / small weights
        qk = pool("qk", 1)       # Qt/Kt per graph
        vpl = pool("vpl", 1)     # V' per graph (node-major, ones cols)
        stp = pool("stp", 1)     # skip per graph
        adjp = pool("adjp", 1)   # adjacency count tiles per graph
        epl = pool("epl", 1)     # exp tiles
        ppl = pool("ppl", 2)     # masked P tiles
        omp = pool("omp", 1)     # attention output chunks per graph
        tmp = pool("tmp", 1)     # small transients
        psA = pool("psA", 2, "PSUM")   # projection psums [128,512]
        psL = pool("psL", 2, "PSUM")   # logits psums
        psO = pool("psO", 2, "PSUM")   # attn-out psums [65,512]
        psB = pool("psB", 2, "PSUM")   # broadcast psums

        def dma(t_ap, src_ap):
            nc.sync.dma_start(t_ap, src_ap)

        lp = ctx.enter_context(nc.allow_low_precision(
            reason="float32r tiles carry full fp32 bits; PE rounds on read"))

        r = f32r  # shorthand
        ones_t = cst.tile([P, P], r, name="ones", tag="ones")
        dma(ones_t[:], ones_d[:].bitcast(r))
        eps_t = cst.tile([P, 1], f32, name="eps", tag="eps")
        nc.vector.memset(eps_t[:], EPS)

        h = [hp.tile([P, NPC], r, name=f"h{c}", tag=f"h{c}") for c in range(4)]
        xt_t = hp.tile([P, NPC], r, name="xt", tag="xt")
        dma(xt_t[:], xt_d[:].bitcast(r))

        # ---------------- GNN layers ----------------
        for l in range(NLAYERS):
            C = IN_CH if l == 0 else HDIM
            CC = C // P
            hin = [xt_t] if l == 0 else h

            wq_t = [wp.tile([P, HDIM], r, name=f"wq{c}", tag=f"wq{c}") for c in range(CC)]
            wk_t = [wp.tile([P, HDIM], r, name=f"wk{c}", tag=f"wk{c}") for c in range(CC)]
            wsk_t = [wp.tile([P, HDIM], r, name=f"wsk{c}", tag=f"wsk{c}") for c in range(CC)]
            wv_t = [wp.tile([P, HDIM], r, name=f"wv{c}", tag=f"wv{c}") for c in range(CC)]
            for c in range(CC):
                dma(wq_t[c][:], wq_d[l][c * P:(c + 1) * P, :].bitcast(r))
                dma(wk_t[c][:], wk_d[l][c * P:(c + 1) * P, :].bitcast(r))
                dma(wsk_t[c][:], wsk_d[l][c * P:(c + 1) * P, :].bitcast(r))
                dma(wv_t[c][:], wv_d[l][c * P:(c + 1) * P, :].bitcast(r))
            ball_t = cst.tile([P, 12], f32, name="ball", tag="ball")
            dma(ball_t[:], ball_d[l][:])
            bvrep_t = cst.tile([P, HDIM], f32, name="bvrep", tag="bvrep")
            dma(bvrep_t[:], bvrep_d[l][:])

            for g in range(GPC):
                col = slice(g * NPG, (g + 1) * NPG)

                adj_t = [adjp.tile([P, NPG], f32, name=f"adj{s}", tag=f"adj{s}")
                         for s in range(4)]
                for s in range(4):
                    dma(adj_t[s][:], adj_d[g, s])

                # Q, K, skip projections (feature-major), bias via ScalarE
                def fm_proj(w_tiles, bcol, opool, otag, odt):
                    outs = []
                    for j in range(4):
                        ps = psA.tile([P, NPG], f32, name="pp", tag="pp")
                        for c in range(CC):
                            nc.tensor.matmul(
                                ps[:], w_tiles[c][:, j * P:(j + 1) * P],
                                hin[c][:, col],
                                start=(c == 0), stop=(c == CC - 1))
                        o = opool.tile([P, NPG], odt, name=f"{otag}{j}", tag=f"{otag}{j}")
                        nc.scalar.activation(
                            o[:], ps[:], AF.Identity,
                            bias=ball_t[:, bcol + j:bcol + j + 1])
                        outs.append(o)
                    return outs

                qt = fm_proj(wq_t, 0, qk, "qt", r)
                kt = fm_proj(wk_t, 4, qk, "kt", r)
                st = fm_proj(wsk_t, 8, stp, "st", f32)

                # V projection (node-major, interleaved ones columns)
                vt = []
                for nb in range(NB):
                    ps = psA.tile([P, HDIM], f32, name="pp", tag="pp")
                    nbl = g * NPG + nb * P
                    for c in range(CC):
                        nc.tensor.matmul(
                            ps[:], hin[c][:, nbl:nbl + P], wv_t[c][:],
                            start=(c == 0), stop=(c == CC - 1))
                    v = vpl.tile([P, HEADS * 65], r, name=f"v{nb}", tag=f"v{nb}")
                    vv = v[:].rearrange("p (h c) -> p h c", c=65)
                    nc.vector.tensor_tensor(
                        vv[:, :, 0:64],
                        ps[:].rearrange("p (h c) -> p h c", c=64),
                        bvrep_t[:].rearrange("p (h c) -> p h c", c=64),
                        OP.add)
                    nc.vector.tensor_copy(
                        vv[:, :, 64:65],
                        ones_t[:, 0:HEADS].rearrange("p (h c) -> p h c", c=1))
                    vt.append(v)

                # attention heads
                om = [omp.tile([P, NPG], f32, name=f"om{c}", tag=f"om{c}") for c in range(4)]
                for hh in range(HEADS):
                    hc, hr = hh // 2, (hh % 2) * 64
                    p_tiles = []
                    for s in range(4):
                        lt = psL.tile([P, NPG], f32, name="lt", tag="lt")
                        nc.tensor.matmul(
                            lt[:], kt[hc][hr:hr + 64, s * P:(s + 1) * P],
                            qt[hc][hr:hr + 64, :], start=True, stop=True)
                        et = epl.tile([P, NPG], f32, name=f"e{s}", tag=f"e{s}")
                        nc.scalar.activation(et[:], lt[:], AF.Exp, scale=0.125)
                        pt = ppl.tile([P, NPG], r, name=f"p{s}", tag=f"p{s}")
                        nc.vector.tensor_tensor(pt[:], et[:], adj_t[s][:],
                                                OP.mult)
                        p_tiles.append(pt)
                    po = psO.tile([65, NPG], f32, name="op", tag="op")
                    for s in range(4):
                        nc.tensor.matmul(
                            po[:], vt[s][:, hh * 65:(hh + 1) * 65],
                            p_tiles[s][:], start=(s == 0), stop=(s == 3))
                    rcp = tmp.tile([65, NPG], r, name="rcp", tag="rcp")
                    nc.vector.reciprocal(rcp[64:65, :], po[64:65, :])
                    bc = psB.tile([64, NPG], f32, name="bc", tag="bc")
                    nc.tensor.matmul(bc[:], ones_t[64:65, 0:64], rcp[64:65, :],
                                     start=True, stop=True)
                    bcs = tmp.tile([64, NPG], f32, name="bcs", tag="bcs")
                    nc.scalar.copy(bcs[:], bc[:])
                    nc.vector.tensor_tensor(
                        om[hc][hr:hr + 64, :], po[0:64, :], bcs[:],
                        OP.mult)

                # skip + relu -> H (in-place column update)
                for c in range(4):
                    sk = tmp.tile([P, NPG], f32, name="sk", tag="sk")
                    nc.gpsimd.tensor_tensor(sk[:], om[c][:], st[c][:], OP.add)
                    nc.scalar.activation(h[c][:, col], sk[:], AF.Relu,
                                         bias=0.0)

        # ---------------- gnn_output Linear ----------------
        wout_t = [cst.tile([P, OUT], r, name=f"wout{c}", tag=f"wout{c}") for c in range(4)]
        for c in range(4):
            dma(wout_t[c][:], wout_d[c * P:(c + 1) * P, :].bitcast(r))
        bout_t = cst.tile([P, 2], f32, name="bout", tag="bout")
        dma(bout_t[:], bout_d[:])
        gt = [hp.tile([P, NPC], r, name="xt", tag="xt"), hp.tile([P, NPC], r, name="gt1", tag="gt1")]
        for j in range(2):
            for n4 in range(4):
                ps = psA.tile([P, NPG], f32, name="pp", tag="pp")
                for c in range(4):
                    nc.tensor.matmul(
                        ps[:], wout_t[c][:, j * P:(j + 1) * P],
                        h[c][:, n4 * NPG:(n4 + 1) * NPG],
                        start=(c == 0), stop=(c == 3))
                nc.scalar.activation(
                    gt[j][:, n4 * NPG:(n4 + 1) * NPG], ps[:], AF.Identity,
                    bias=bout_t[:, j:j + 1])

        # ---------------- global MHA + LayerNorm ----------------
        wqkv_t = [cst.tile([P, 3 * OUT], r, name=f"wqkv{c}", tag=f"wqkv{c}") for c in range(2)]
        for c in range(2):
            dma(wqkv_t[c][:], wqkv_d[c * P:(c + 1) * P, :].bitcast(r))
        wop_t = [cst.tile([64, OUT], r, name=f"wop{c}", tag=f"wop{c}")
                 for c in range(4)]
        for c in range(4):
            dma(wop_t[c][:], wop_d[c * 64:(c + 1) * 64, :].bitcast(r))
        bqk_t = cst.tile([P, 4], f32, name="bqk", tag="bqk")
        dma(bqk_t[:], bqk_d[:])
        bvh_t = cst.tile([P, OUT], f32, name="bvh", tag="bvh")
        dma(bvh_t[:], bvh_d[:])
        bop_t = cst.tile([P, OUT], f32, name="bop", tag="bop")
        dma(bop_t[:], bop_d[:])
        lng_t = cst.tile([P, OUT], f32, name="lng", tag="lng")
        dma(lng_t[:], lng_d[:])
        lnb_t = cst.tile([P, OUT], f32, name="lnb", tag="lnb")
        dma(lnb_t[:], lnb_d[:])

        isq = 1.0 / np.sqrt(OUT // HEADS)

        for g in range(GPC):
            col = slice(g * NPG, (g + 1) * NPG)

            # Q-hat / K-hat (feature-major, 64-partition chunks so per-head
            # row offsets stay in {0, 32})
            qkt = []
            for j in range(4):
                ps = psA.tile([P, NPG], f32, name="pp", tag="pp")
                for c in range(2):
                    nc.tensor.matmul(
                        ps[:], wqkv_t[c][:, j * P:(j + 1) * P],
                        gt[c][:, col], start=(c == 0), stop=(c == 1))
                base = "qt" if j < 2 else "kt"
                for half in range(2):
                    nm = f"{base}64_{(j % 2) * 2 + half}"
                    o = qk.tile([64, NPG], r, name=nm, tag=nm)
                    nc.scalar.activation(
                        o[:], ps[half * 64:(half + 1) * 64, :], AF.Identity,
                        bias=bqk_t[half * 64:(half + 1) * 64, j:j + 1])
                    qkt.append(o)
            qht, kht = qkt[0:4], qkt[4:8]

            # V-hat (node-major, ones cols, 8 heads x 33)
            vht = []
            for nb in range(NB):
                ps = psA.tile([P, OUT], f32, name="pp", tag="pp")
                nbl = g * NPG + nb * P
                for c in range(2):
                    nc.tensor.matmul(
                        ps[:], gt[c][:, nbl:nbl + P],
                        wqkv_t[c][:, 2 * OUT:3 * OUT],
                        start=(c == 0), stop=(c == 1))
                v = vpl.tile([P, HEADS * 33], r, name=f"v{nb}", tag=f"v{nb}")
                vv = v[:].rearrange("p (h c) -> p h c", c=33)
                nc.vector.tensor_tensor(
                    vv[:, :, 0:32],
                    ps[:].rearrange("p (h c) -> p h c", c=32),
                    bvh_t[:].rearrange("p (h c) -> p h c", c=32),
                    OP.add)
                nc.vector.tensor_copy(
                    vv[:, :, 32:33],
                    ones_t[:, 0:HEADS].rearrange("p (h c) -> p h c", c=1))
                vht.append(v)

            omh = [omp.tile([64, NPG], r, name=f"omh{c}", tag=f"om{c}")
                   for c in range(4)]
            for hh in range(HEADS):
                hc, hr = hh // 2, (hh % 2) * 32
                e_tiles = []
                for s in range(4):
                    lt = psL.tile([P, NPG], f32, name="lt", tag="lt")
                    nc.tensor.matmul(
                        lt[:], kht[hc][hr:hr + 32, s * P:(s + 1) * P],
                        qht[hc][hr:hr + 32, :], start=True, stop=True)
                    et = ppl.tile([P, NPG], r, name=f"p{s}", tag=f"p{s}")
                    nc.scalar.activation(et[:], lt[:], AF.Exp, scale=isq)
                    e_tiles.append(et)
                po = psO.tile([65, NPG], f32, name="op", tag="op")
                for s in range(4):
                    nc.tensor.matmul(
                        po[0:33, :], vht[s][:, hh * 33:(hh + 1) * 33],
                        e_tiles[s][:], start=(s == 0), stop=(s == 3))
                rcp = tmp.tile([65, NPG], r, name="rcp", tag="rcp")
                nc.vector.reciprocal(rcp[32:33, :], po[32:33, :])
                bc = psB.tile([64, NPG], f32, name="bc", tag="bc")
                nc.tensor.matmul(bc[0:32, :], ones_t[32:33, 0:32],
                                 rcp[32:33, :], start=True, stop=True)
                bcs = tmp.tile([64, NPG], f32, name="bcs", tag="bcs")
                nc.scalar.copy(bcs[0:32, :], bc[0:32, :])
                nc.vector.tensor_tensor(
                    omh[hc][hr:hr + 32, :], po[0:32, :], bcs[0:32, :],
                    OP.mult)

            # out-proj + LayerNorm (node-major)
            for nb in range(NB):
                ps = psA.tile([P, OUT], f32, name="pp", tag="pp")
                nbl = nb * P
                for c in range(4):
                    nc.tensor.matmul(
                        ps[:], omh[c][:, nbl:nbl + P], wop_t[c][:],
                        start=(c == 0), stop=(c == 3))
                xs = tmp.tile([P, OUT], f32, name="xs", tag="xs")
                nc.vector.tensor_tensor(xs[:], ps[:], bop_t[:], OP.add)
                s1 = tmp.tile([P, 1], f32, name="s1", tag="s1")
                nc.vector.tensor_reduce(s1[:], xs[:], mybir.AxisListType.X,
                                        OP.add)
                mu = tmp.tile([P, 1], f32, name="mu", tag="mu")
                nc.vector.tensor_scalar_mul(mu[:], s1[:], 1.0 / OUT)
                zc = tmp.tile([P, OUT], f32, name="zc", tag="zc")
                nc.vector.tensor_scalar_sub(zc[:], xs[:], mu[:])
                sq = tmp.tile([P, OUT], f32, name="sq", tag="sq")
                nc.vector.tensor_tensor(sq[:], zc[:], zc[:], OP.mult)
                s2 = tmp.tile([P, 1], f32, name="s2", tag="s2")
                nc.vector.tensor_reduce(s2[:], sq[:], mybir.AxisListType.X,
                                        OP.add)
                sd = tmp.tile([P, 1], f32, name="sd", tag="sd")
                nc.scalar.activation(sd[:], s2[:], AF.Sqrt, scale=1.0 / OUT,
                                     bias=eps_t[:])
                rs = tmp.tile([P, 1], f32, name="rs", tag="rs")
                nc.vector.reciprocal(rs[:], sd[:])
                y1 = tmp.tile([P, OUT], f32, name="y1", tag="y1")
                nc.vector.scalar_tensor_tensor(y1[:], zc[:], rs[:], lng_t[:],
                                               OP.mult, OP.mult)
                fin = tmp.tile([P, OUT], f32, name="fin", tag="fin")
                nc.gpsimd.tensor_tensor(fin[:], y1[:], lnb_t[:], OP.add)
                dma(out_d[g * NPG + nbl:g * NPG + nbl + P, :], fin[:])

    nc.compile()
    return nc


def _prep_inputs(x, edge_index, batch, params):
    """Host-side shard + layout prep. Returns per-core input maps."""
    x = np.asarray(x, dtype=np.float32)
    edge_index = np.asarray(edge_index)
    src, dst = edge_index[0].astype(np.int64), edge_index[1].astype(np.int64)

    # per-graph edge-count matrices A[g, src_local, dst_local]
    gid = dst // NPG
    idx = gid * (NPG * NPG) + (src % NPG) * NPG + (dst % NPG)
    A = np.bincount(idx, minlength=NG * NPG * NPG).astype(np.float32)
    A = A.reshape(NG, NPG, NPG)

    L = params["layers"]

    def chunks12(l):
        out = np.empty((P, 12), np.float32)
        for t, nm in enumerate(("q", "k", "skip")):
            b = np.asarray(L[l]["b" + nm], np.float32)
            for c in range(4):
                out[:, t * 4 + c] = b[c * P:(c + 1) * P]
        return out

    common = {"ones": np.ones((P, P), np.float32)}
    for l in range(NLAYERS):
        common[f"wq{l}"] = np.asarray(L[l]["Wq"], np.float32)
        common[f"wk{l}"] = np.asarray(L[l]["Wk"], np.float32)
        common[f"wsk{l}"] = np.asarray(L[l]["Wskip"], np.float32)
        common[f"wv{l}"] = np.asarray(L[l]["Wv"], np.float32)
        common[f"ball{l}"] = chunks12(l)
        common[f"bvrep{l}"] = np.tile(
            np.asarray(L[l]["bv"], np.float32)[None, :], (P, 1))
    common["wout"] = np.asarray(params["Wout"], np.float32)
    bout = np.asarray(params["bout"], np.float32)
    common["bout"] = np.stack([bout[0:P], bout[P:2 * P]], axis=1)
    common["wqkv"] = np.ascontiguousarray(
        np.asarray(params["in_proj_w"], np.float32).T)
    ipb = np.asarray(params["in_proj_b"], np.float32)
    common["bqk"] = np.stack([ipb[0:P], ipb[P:2 * P],
                              ipb[2 * P:3 * P], ipb[3 * P:4 * P]], axis=1)
    common["bvh"] = np.tile(ipb[2 * OUT:3 * OUT][None, :], (P, 1))
    common["wop"] = np.ascontiguousarray(
        np.asarray(params["out_proj_w"], np.float32).T)
    common["bop"] = np.tile(
        np.asarray(params["out_proj_b"], np.float32)[None, :], (P, 1))
    common["lng"] = np.tile(np.asarray(params["ln_g"], np.float32)[None, :],
                            (P, 1))
    common["lnb"] = np.tile(np.asarray(params["ln_b"], np.float32)[None, :],
                            (P, 1))

    in_maps = []
    for c in range(NCORES):
        m = dict(common)
        xs = x[c * NPC:(c + 1) * NPC, :]                    # [NPC, IN_CH]
        m["xt"] = np.ascontiguousarray(xs.T)                # [128, NPC]
        Ac = A[c * GPC:(c + 1) * GPC]                       # [GPC, 512, 512]
        m["adj"] = np.ascontiguousarray(
            Ac.reshape(GPC, 4, P, NPG))                     # src-chunk-major
        in_maps.append(m)
    return in_maps


def _get_nc():
    if "nc" not in _cache:
        _cache["nc"] = _build()
    return _cache["nc"]


def kernel(x, edge_index, batch, params):
    from concourse.bass_utils import run_bass_kernel_spmd

    nc = _get_nc()
    in_maps = _prep_inputs(x, edge_index, batch, params)
    res = run_bass_kernel_spmd(nc, in_maps, list(range(NCORES)))
    out = np.concatenate([res.results[c]["out"] for c in range(NCORES)],
                         axis=0)
    return out.astype(np.float32)


# revision 32
# speedup vs baseline: 5.7175x; 5.7175x over previous
# Trainium2 Bass kernel for EnhancedGNNTransformerEncoder.
#
# Strategy (data-parallel over graphs, 4 graphs/core on 8 cores):
# The per-graph sparse TransformerConv attention (16 random in-edges per node,
# edges never cross graphs) is reformulated as dense per-graph 512x512
# attention with an edge-count mask A[src,dst] (host-precomputed from
# edge_index; exactly reproduces segment-softmax semantics incl. duplicate
# edges).  Everything on-device is dense linear algebra:
#   per layer:  Q/K/skip projections (feature-major), V projection
#   (node-major, bf16, with interleaved ones columns that produce softmax
#   denominators for free), per-(graph,head): Lt = Kt_h^T @ Qt_h on PE
#   (float32r), exp on ScalarE (bf16 out), count-mask multiply on VectorE
#   (bf16), P @ [V|1] on PE (bf16), then reciprocal + PE-broadcast + multiply
#   for the softmax division, skip-add and ReLU on GpSimd.  Then gnn_output
#   Linear, per-graph dense MHA (same pattern, no mask), out-proj fused with
#   LayerNorm on VectorE.
# Matmul activations/weights are float32r (1 cycle/row, ~1e-4 rel err) except
# the probability @ value matmuls which are bf16; accumulation always fp32.
# Softmax max-subtraction is dropped: logits are O(10), exp cannot overflow,
# and normalization makes it mathematically identical.

import sys

for _p in ("/opt/trn_rl_repo", "/opt/trn_rl_repo/concourse"):
    if _p not in sys.path:
        sys.path.insert(0, _p)

import numpy as np

NG, NPG, DEG = 32, 512, 16
N, E = NG * NPG, NG * NPG * DEG
IN_CH, HID, HEADS, HDIM, OUT, NLAYERS = 128, 64, 8, 512, 256, 4
NCORES = 8
GPC = NG // NCORES          # graphs per core
NPC = GPC * NPG             # nodes per core
P = 128
NB = NPG // P               # 128-node blocks per graph
EPS = 1e-5

_cache = {}


def _build():
    import concourse.bass as bass  # noqa: F401
    import concourse.tile as tile
    from concourse import bacc, mybir
    from contextlib import ExitStack

    f32, f32r = mybir.dt.float32, mybir.dt.float32r
    AF = mybir.ActivationFunctionType
    OP = mybir.AluOpType

    nc = bacc.Bacc("TRN2", target_bir_lowering=False, debug=False,
                   num_devices=NCORES)

    def din(name, shape):
        return nc.dram_tensor(name, shape, f32, kind="ExternalInput").ap()

    xt_d = din("xt", [P, NPC])
    adj_d = nc.dram_tensor("adj", [GPC, P, 4 * NPG], mybir.dt.bfloat16,
                           kind="ExternalInput").ap()
    ones_d = din("ones", [P, P])
    wq_d, wk_d, wsk_d, wv_d, ball_d, bvrep_d = [], [], [], [], [], []
    for l in range(NLAYERS):
        C = IN_CH if l == 0 else HDIM
        wq_d.append(din(f"wq{l}", [C, HDIM]))
        wk_d.append(din(f"wk{l}", [C, HDIM]))
        wsk_d.append(din(f"wsk{l}", [C, HDIM]))
        wv_d.append(din(f"wv{l}", [C, HDIM]))
        ball_d.append(din(f"ball{l}", [P, 12]))
        bvrep_d.append(din(f"bvrep{l}", [P, HDIM]))
    wout_d = din("wout", [HDIM, OUT])
    bout_d = din("bout", [P, 2])
    wqkv_d = din("wqkv", [OUT, 3 * OUT])
    bqk_d = din("bqk", [P, 4])
    bvh_d = din("bvh", [P, OUT])
    wop_d = din("wop", [OUT, OUT])
    bop_d = din("bop", [P, OUT])
    lng_d = din("lng", [P, OUT])
    lnb_d = din("lnb", [P, OUT])
    out_d = nc.dram_tensor("out", [NPC, OUT], f32, kind="ExternalOutput").ap()

    with tile.TileContext(nc) as tc, ExitStack() as ctx:
        def pool(name, bufs, space="SBUF"):
            return ctx.enter_context(
                tc.tile_pool(name=name, bufs=bufs, space=space))

        hp = pool("hp", 1)       # H chunks, xt, GT
        wp = pool("wp", 1)       # layer weights
        cst = pool("cst", 1)     # constants / small weights
        qk = pool("qk", 1)       # Qt/Kt per graph
        vpl = pool("vpl", 2)     # V' per graph (node-major, ones cols)
        stp = pool("stp", 1)     # skip per graph
        adjp = pool("adjp", 1)   # adjacency count tiles per graph
        epl = pool("epl", 2)     # exp tiles
        ppl = pool("ppl", 2)     # masked P tiles
        omp = pool("omp", 2)     # attention output chunks per graph
        tmp = pool("tmp", 2)     # small transients
        psA = pool("psA", 2, "PSUM")   # projection + broadcast psums
        psL = pool("psL", 2, "PSUM")   # logits psums [128, 2*512]
        psO = pool("psO", 2, "PSUM")   # attn-out psums [65,512]

        def dma(t_ap, src_ap):
            nc.sync.dma_start(t_ap, src_ap)

        lp = ctx.enter_context(nc.allow_low_precision(
            reason="float32r tiles carry full fp32 bits; PE rounds on read"))

        r = f32r  # shorthand
        bf = mybir.dt.bfloat16
        ones_t = cst.tile([P, P], r, name="ones", tag="ones")
        dma(ones_t[:], ones_d[:].bitcast(r))
        eps_t = cst.tile([P, 1], f32, name="eps", tag="eps")
        nc.vector.memset(eps_t[:], EPS)
        onesb_t = cst.tile([P, HEADS], bf, name="onesb", tag="onesb")
        nc.vector.memset(onesb_t[:], 1.0)

        h = [hp.tile([P, NPC], r, name=f"h{c}", tag=f"h{c}") for c in range(4)]
        xt_t = hp.tile([P, NPC], r, name="xt", tag="xt")
        dma(xt_t[:], xt_d[:].bitcast(r))

        # ---------------- GNN layers ----------------
        for l in range(NLAYERS):
            C = IN_CH if l == 0 else HDIM
            CC = C // P
            hin = [xt_t] if l == 0 else h

            wq_t = [wp.tile([P, HDIM], r, name=f"wq{c}", tag=f"wq{c}") for c in range(CC)]
            wk_t = [wp.tile([P, HDIM], r, name=f"wk{c}", tag=f"wk{c}") for c in range(CC)]
            wsk_t = [wp.tile([P, HDIM], r, name=f"wsk{c}", tag=f"wsk{c}") for c in range(CC)]
            wv_t = [wp.tile([P, HDIM], r, name=f"wv{c}", tag=f"wv{c}") for c in range(CC)]
            for c in range(CC):
                dma(wq_t[c][:], wq_d[l][c * P:(c + 1) * P, :].bitcast(r))
                dma(wk_t[c][:], wk_d[l][c * P:(c + 1) * P, :].bitcast(r))
                dma(wsk_t[c][:], wsk_d[l][c * P:(c + 1) * P, :].bitcast(r))
                dma(wv_t[c][:], wv_d[l][c * P:(c + 1) * P, :].bitcast(r))
            ball_t = cst.tile([P, 12], f32, name="ball", tag="ball")
            dma(ball_t[:], ball_d[l][:])
            bvrep_t = cst.tile([P, HDIM], f32, name="bvrep", tag="bvrep")
            dma(bvrep_t[:], bvrep_d[l][:])

            for g in range(GPC):
                col = slice(g * NPG, (g + 1) * NPG)

                adjb_t = adjp.tile([P, 4 * NPG], bf, name="adjb", tag="adjb")
                dma(adjb_t[:], adj_d[g])

                # Q, K, skip projections (feature-major), bias via ScalarE
                def fm_proj(w_tiles, bcol, opool, otag, odt, engine="act"):
                    outs = []
                    for j in range(4):
                        ps = psA.tile([P, NPG], f32, name="pp", tag="pp")
                        for c in range(CC):
                            nc.tensor.matmul(
                                ps[:], w_tiles[c][:, j * P:(j + 1) * P],
                                hin[c][:, col],
                                start=(c == 0), stop=(c == CC - 1))
                        o = opool.tile([P, NPG], odt, name=f"{otag}{j}", tag=f"{otag}{j}")
                        bias_ap = ball_t[:, bcol + j:bcol + j + 1]
                        if engine == "act":
                            nc.scalar.activation(o[:], ps[:], AF.Identity,
                                                 bias=bias_ap)
                        else:
                            nc.vector.tensor_scalar_add(o[:], ps[:], bias_ap)
                        outs.append(o)
                    return outs

                qt = fm_proj(wq_t, 0, qk, "qt", r)
                kt = fm_proj(wk_t, 4, qk, "kt", r)
                st = fm_proj(wsk_t, 8, stp, "st", f32, engine="dve")

                # V projection (node-major, interleaved ones columns)
                vt = []
                for nb in range(NB):
                    ps = psA.tile([P, HDIM], f32, name="pp", tag="pp")
                    nbl = g * NPG + nb * P
                    for c in range(CC):
                        nc.tensor.matmul(
                            ps[:], hin[c][:, nbl:nbl + P], wv_t[c][:],
                            start=(c == 0), stop=(c == CC - 1))
                    v = vpl.tile([P, HEADS * 65], bf, name=f"v{nb}", tag=f"v{nb}")
                    vv = v[:].rearrange("p (h c) -> p h c", c=65)
                    nc.vector.tensor_tensor(
                        vv[:, :, 0:64],
                        ps[:].rearrange("p (h c) -> p h c", c=64),
                        bvrep_t[:].rearrange("p (h c) -> p h c", c=64),
                        OP.add)
                    nc.gpsimd.tensor_copy(
                        vv[:, :, 64:65],
                        onesb_t[:].rearrange("p (h c) -> p h c", c=1))
                    vt.append(v)

                # attention heads: fused [128, 4*NPG] logits psum, single
                # exp (bf16 out), single bf16 mask-mul, bf16 O-matmuls.
                om = [omp.tile([P, NPG], f32, name=f"om{c}", tag=f"om{c}") for c in range(4)]
                for hh in range(HEADS):
                    hc, hr = hh // 2, (hh % 2) * 64
                    pt = ppl.tile([P, 4 * NPG], bf, name="p0", tag="p0")
                    for hf in range(2):
                        lt = psL.tile([P, 2 * NPG], f32, name="lt", tag="lt")
                        for s2 in range(2):
                            s = hf * 2 + s2
                            nc.tensor.matmul(
                                lt[:, s2 * NPG:(s2 + 1) * NPG],
                                kt[hc][hr:hr + 64, s * P:(s + 1) * P],
                                qt[hc][hr:hr + 64, :], start=True, stop=True)
                        et = epl.tile([P, 2 * NPG], bf, name="e0", tag="e0")
                        nc.scalar.activation(et[:], lt[:], AF.Exp, scale=0.125)
                        nc.vector.tensor_tensor(
                            pt[:, hf * 2 * NPG:(hf + 1) * 2 * NPG], et[:],
                            adjb_t[:, hf * 2 * NPG:(hf + 1) * 2 * NPG],
                            OP.mult)
                    po = psO.tile([65, NPG], f32, name="op", tag="op")
                    for s in range(4):
                        nc.tensor.matmul(
                            po[:], vt[s][:, hh * 65:(hh + 1) * 65],
                            pt[:, s * NPG:(s + 1) * NPG],
                            start=(s == 0), stop=(s == 3))
                    rcp = tmp.tile([65, NPG], r, name="rcp", tag="rcp")
                    nc.vector.reciprocal(rcp[64:65, :], po[64:65, :])
                    bc = psA.tile([64, NPG], f32, name="bc", tag="pp")
                    nc.tensor.matmul(bc[:], ones_t[64:65, 0:64],
                                     rcp[64:65, :], start=True, stop=True)
                    bcs = tmp.tile([64, NPG], f32, name="bcs", tag="bcs")
                    nc.scalar.copy(bcs[:], bc[:])
                    nc.vector.tensor_tensor(
                        om[hc][hr:hr + 64, :], po[0:64, :], bcs[:],
                        OP.mult)

                # skip + relu -> H (in-place column update)
                for c in range(4):
                    sk = tmp.tile([P, NPG], f32, name="sk", tag="sk")
                    nc.gpsimd.tensor_tensor(sk[:], om[c][:], st[c][:], OP.add)
                    nc.gpsimd.tensor_scalar_max(h[c][:, col], sk[:], 0.0)

        # ---------------- gnn_output Linear ----------------
        wout_t = [cst.tile([P, OUT], r, name=f"wout{c}", tag=f"wout{c}") for c in range(4)]
        for c in range(4):
            dma(wout_t[c][:], wout_d[c * P:(c + 1) * P, :].bitcast(r))
        bout_t = cst.tile([P, 2], f32, name="bout", tag="bout")
        dma(bout_t[:], bout_d[:])
        gt = [hp.tile([P, NPC], r, name="xt", tag="xt"),
              hp.tile([P, NPC], r, name="gt1", tag="gt1")]
        for j in range(2):
            for n4 in range(4):
                ps = psA.tile([P, NPG], f32, name="pp", tag="pp")
                for c in range(4):
                    nc.tensor.matmul(
                        ps[:], wout_t[c][:, j * P:(j + 1) * P],
                        h[c][:, n4 * NPG:(n4 + 1) * NPG],
                        start=(c == 0), stop=(c == 3))
                nc.scalar.activation(
                    gt[j][:, n4 * NPG:(n4 + 1) * NPG], ps[:], AF.Identity,
                    bias=bout_t[:, j:j + 1])

        # ---------------- global MHA + LayerNorm ----------------
        wqkv_t = [cst.tile([P, 3 * OUT], r, name=f"wqkv{c}", tag=f"wqkv{c}") for c in range(2)]
        for c in range(2):
            dma(wqkv_t[c][:], wqkv_d[c * P:(c + 1) * P, :].bitcast(r))
        wop_t = [cst.tile([64, OUT], r, name=f"wop{c}", tag=f"wop{c}")
                 for c in range(4)]
        for c in range(4):
            dma(wop_t[c][:], wop_d[c * 64:(c + 1) * 64, :].bitcast(r))
        bqk_t = cst.tile([P, 4], f32, name="bqk", tag="bqk")
        dma(bqk_t[:], bqk_d[:])
        bvh_t = cst.tile([P, OUT], f32, name="bvh", tag="bvh")
        dma(bvh_t[:], bvh_d[:])
        bop_t = cst.tile([P, OUT], f32, name="bop", tag="bop")
        dma(bop_t[:], bop_d[:])
        lng_t = cst.tile([P, OUT], f32, name="lng", tag="lng")
        dma(lng_t[:], lng_d[:])
        lnb_t = cst.tile([P, OUT], f32, name="lnb", tag="lnb")
        dma(lnb_t[:], lnb_d[:])

        isq = 1.0 / np.sqrt(OUT // HEADS)

        for g in range(GPC):
            col = slice(g * NPG, (g + 1) * NPG)

            # Q-hat / K-hat (feature-major, 64-partition chunks so per-head
            # row offsets stay in {0, 32})
            qkt = []
            for j in range(4):
                ps = psA.tile([P, NPG], f32, name="pp", tag="pp")
                for c in range(2):
                    nc.tensor.matmul(
                        ps[:], wqkv_t[c][:, j * P:(j + 1) * P],
                        gt[c][:, col], start=(c == 0), stop=(c == 1))
                base = "qt" if j < 2 else "kt"
                for half in range(2):
                    nm = f"{base}64_{(j % 2) * 2 + half}"
                    o = qk.tile([64, NPG], r, name=nm, tag=nm)
                    nc.scalar.activation(
                        o[:], ps[half * 64:(half + 1) * 64, :], AF.Identity,
                        bias=bqk_t[half * 64:(half + 1) * 64, j:j + 1])
                    qkt.append(o)
            qht, kht = qkt[0:4], qkt[4:8]

            # V-hat (node-major, ones cols, 8 heads x 33)
            vht = []
            for nb in range(NB):
                ps = psA.tile([P, OUT], f32, name="pp", tag="pp")
                nbl = g * NPG + nb * P
                for c in range(2):
                    nc.tensor.matmul(
                        ps[:], gt[c][:, nbl:nbl + P],
                        wqkv_t[c][:, 2 * OUT:3 * OUT],
                        start=(c == 0), stop=(c == 1))
                v = vpl.tile([P, HEADS * 33], bf, name=f"v{nb}", tag=f"v{nb}")
                vv = v[:].rearrange("p (h c) -> p h c", c=33)
                nc.vector.tensor_tensor(
                    vv[:, :, 0:32],
                    ps[:].rearrange("p (h c) -> p h c", c=32),
                    bvh_t[:].rearrange("p (h c) -> p h c", c=32),
                    OP.add)
                nc.gpsimd.tensor_copy(
                    vv[:, :, 32:33],
                    onesb_t[:].rearrange("p (h c) -> p h c", c=1))
                vht.append(v)

            omh = [omp.tile([64, NPG], r, name=f"omh{c}", tag=f"om{c}")
                   for c in range(4)]
            for hh in range(HEADS):
                hc, hr = hh // 2, (hh % 2) * 32
                et = ppl.tile([P, 4 * NPG], bf, name="p0", tag="p0")
                for hf in range(2):
                    lt = psL.tile([P, 2 * NPG], f32, name="lt", tag="lt")
                    for s2 in range(2):
                        s = hf * 2 + s2
                        nc.tensor.matmul(
                            lt[:, s2 * NPG:(s2 + 1) * NPG],
                            kht[hc][hr:hr + 32, s * P:(s + 1) * P],
                            qht[hc][hr:hr + 32, :], start=True, stop=True)
                    nc.scalar.activation(
                        et[:, hf * 2 * NPG:(hf + 1) * 2 * NPG], lt[:],
                        AF.Exp, scale=isq)
                po = psO.tile([65, NPG], f32, name="op", tag="op")
                for s in range(4):
                    nc.tensor.matmul(
                        po[0:33, :], vht[s][:, hh * 33:(hh + 1) * 33],
                        et[:, s * NPG:(s + 1) * NPG],
                        start=(s == 0), stop=(s == 3))
                rcp = tmp.tile([65, NPG], r, name="rcp", tag="rcp")
                nc.vector.reciprocal(rcp[32:33, :], po[32:33, :])
                bc = psA.tile([64, NPG], f32, name="bc", tag="pp")
                nc.tensor.matmul(bc[0:32, :], ones_t[32:33, 0:32],
                                 rcp[32:33, :], start=True, stop=True)
                bcs = tmp.tile([64, NPG], f32, name="bcs", tag="bcs")
                nc.scalar.copy(bcs[0:32, :], bc[0:32, :])
                nc.vector.tensor_tensor(
                    omh[hc][hr:hr + 32, :], po[0:32, :], bcs[0:32, :],
                    OP.mult)

            # out-proj + LayerNorm (node-major)
            for nb in range(NB):
                ps = psA.tile([P, OUT], f32, name="pp", tag="pp")
                nbl = nb * P
                for c in range(4):
                    nc.tensor.matmul(
                        ps[:], omh[c][:, nbl:nbl + P], wop_t[c][:],
                        start=(c == 0), stop=(c == 3))
                xs = tmp.tile([P, OUT], f32, name="xs", tag="xs", bufs=1)
                nc.vector.tensor_tensor(xs[:], ps[:], bop_t[:], OP.add)
                s1 = tmp.tile([P, 1], f32, name="s1", tag="s1", bufs=1)
                nc.vector.tensor_reduce(s1[:], xs[:], mybir.AxisListType.X,
                                        OP.add)
                mu = tmp.tile([P, 1], f32, name="mu", tag="mu", bufs=1)
                nc.vector.tensor_scalar_mul(mu[:], s1[:], 1.0 / OUT)
                zc = tmp.tile([P, OUT], f32, name="zc", tag="zc", bufs=1)
                nc.vector.tensor_scalar_sub(zc[:], xs[:], mu[:])
                sq = tmp.tile([P, OUT], f32, name="sq", tag="sq", bufs=1)
                nc.vector.tensor_tensor(sq[:], zc[:], zc[:], OP.mult)
                s2 = tmp.tile([P, 1], f32, name="s2", tag="s2", bufs=1)
                nc.vector.tensor_reduce(s2[:], sq[:], mybir.AxisListType.X,
                                        OP.add)
                sd = tmp.tile([P, 1], f32, name="sd", tag="sd", bufs=1)
                nc.scalar.activation(sd[:], s2[:], AF.Sqrt, scale=1.0 / OUT,
                                     bias=eps_t[:])
                rs = tmp.tile([P, 1], f32, name="rs", tag="rs", bufs=1)
                nc.vector.reciprocal(rs[:], sd[:])
                y1 = tmp.tile([P, OUT], f32, name="y1", tag="y1", bufs=1)
                nc.vector.scalar_tensor_tensor(y1[:], zc[:], rs[:], lng_t[:],
                                               OP.mult, OP.mult)
                fin = tmp.tile([P, OUT], f32, name="fin", tag="fin", bufs=1)
                nc.gpsimd.tensor_tensor(fin[:], y1[:], lnb_t[:], OP.add)
                dma(out_d[g * NPG + nbl:g * NPG + nbl + P, :], fin[:])

    nc.compile()
    return nc


def _prep_inputs(x, edge_index, batch, params):
    """Host-side shard + layout prep. Returns per-core input maps."""
    x = np.asarray(x, dtype=np.float32)
    edge_index = np.asarray(edge_index)
    src, dst = edge_index[0].astype(np.int64), edge_index[1].astype(np.int64)

    # per-graph edge-count matrices A[g, src_local, dst_local]
    gid = dst // NPG
    idx = gid * (NPG * NPG) + (src % NPG) * NPG + (dst % NPG)
    A = np.bincount(idx, minlength=NG * NPG * NPG).astype(np.float32)
    A = A.reshape(NG, NPG, NPG)

    L = params["layers"]

    def chunks12(l):
        out = np.empty((P, 12), np.float32)
        for t, nm in enumerate(("q", "k", "skip")):
            b = np.asarray(L[l]["b" + nm], np.float32)
            for c in range(4):
                out[:, t * 4 + c] = b[c * P:(c + 1) * P]
        return out

    common = {"ones": np.ones((P, P), np.float32)}
    for l in range(NLAYERS):
        common[f"wq{l}"] = np.asarray(L[l]["Wq"], np.float32)
        common[f"wk{l}"] = np.asarray(L[l]["Wk"], np.float32)
        common[f"wsk{l}"] = np.asarray(L[l]["Wskip"], np.float32)
        common[f"wv{l}"] = np.asarray(L[l]["Wv"], np.float32)
        common[f"ball{l}"] = chunks12(l)
        common[f"bvrep{l}"] = np.tile(
            np.asarray(L[l]["bv"], np.float32)[None, :], (P, 1))
    common["wout"] = np.asarray(params["Wout"], np.float32)
    bout = np.asarray(params["bout"], np.float32)
    common["bout"] = np.stack([bout[0:P], bout[P:2 * P]], axis=1)
    common["wqkv"] = np.ascontiguousarray(
        np.asarray(params["in_proj_w"], np.float32).T)
    ipb = np.asarray(params["in_proj_b"], np.float32)
    common["bqk"] = np.stack([ipb[0:P], ipb[P:2 * P],
                              ipb[2 * P:3 * P], ipb[3 * P:4 * P]], axis=1)
    common["bvh"] = np.tile(ipb[2 * OUT:3 * OUT][None, :], (P, 1))
    common["wop"] = np.ascontiguousarray(
        np.asarray(params["out_proj_w"], np.float32).T)
    common["bop"] = np.tile(
        np.asarray(params["out_proj_b"], np.float32)[None, :], (P, 1))
    common["lng"] = np.tile(np.asarray(params["ln_g"], np.float32)[None, :],
                            (P, 1))
    common["lnb"] = np.tile(np.asarray(params["ln_b"], np.float32)[None, :],
                            (P, 1))

    in_maps = []
    for c in range(NCORES):
        m = dict(common)
        xs = x[c * NPC:(c + 1) * NPC, :]                    # [NPC, IN_CH]
        m["xt"] = np.ascontiguousarray(xs.T)                # [128, NPC]
        import ml_dtypes
        Ac = A[c * GPC:(c + 1) * GPC]                       # [GPC, 512, 512]
        m["adj"] = np.ascontiguousarray(
            Ac.reshape(GPC, 4, P, NPG).transpose(0, 2, 1, 3)
            .reshape(GPC, P, 4 * NPG).astype(ml_dtypes.bfloat16))
        in_maps.append(m)
    return in_maps


def _get_nc():
    if "nc" not in _cache:
        _cache["nc"] = _build()
    return _cache["nc"]


def kernel(x, edge_index, batch, params):
    from concourse.bass_utils import run_bass_kernel_spmd

    nc = _get_nc()
    in_maps = _prep_inputs(x, edge_index, batch, params)
    res = run_bass_kernel_spmd(nc, in_maps, list(range(NCORES)))
    out = np.concatenate([res.results[c]["out"] for c in range(NCORES)],
                         axis=0)
    return out.astype(np.float32)
